# revision 1
# baseline (speedup 1.0000x reference)
"""AdderConv (AdderNet conv 3x3 + BatchNorm2d, training stats) on 8 trn2 cores.

Reference computation:
  u[n,o,yx] = sum_{c,dy,dx} |x[n,c,y+dy-1,x+dx-1] - W[o,c,dy,dx]|   (zero padded)
  out = -u, then BatchNorm2d over (n, y, x) per channel o with affine gamma/beta.

Sharding: output channels. Core k owns channels [8k, 8k+8); every core reads the
full x. BatchNorm stats are per-channel, hence fully core-local (no collectives).

Key algebra: |x - w| = x + w - 2*min(x, w).
  u[o,s] = S_x(s) + S_w(o) - 2 * sum_k min(x_k(s), w_ok)
  - S_w(o) is constant per channel -> shift-invariant under BatchNorm -> dropped.
  - S_x(s) = sum_{c,j in min-taps} x[c, s+d_j] is channel-independent: a 3x3
    box filter of the channel-summed input (0.1% of the kernel's FLOPs). It is
    precomputed host-side alongside the input layout prep and accumulated into
    the PSUM group by a K=1 ones matmul.
  - min(x, w) is ONE stock DVE tensor_scalar(op0=min) per tap: bf16 4x mode.
    (TRN2's DVE has no abs ALU op, so a direct |x-w| needs >=2 DVE passes.)
  - Taps j=4 (both groups) and j=1 (group 0) run on the Scalar engine instead
    as fused |x + (-w)| via activation(func=Abs, bias=-w), accumulated with +G
    and excluded from S_x, balancing DVE vs ACT load.

Data staging (the SP DMA sequencer costs ~0.7us per dynamic DMA, so DMA count
is the scarce resource): x is pre-padded/replicated/bf16-cast on the host into
xpad[8, 128, 960] (partitions = 4 o-slots x 32 channels, zero borders, row
stride 32) plus a one-element-shifted copy xodd (keeps dx=1 windows 4-byte
aligned for the DVE packed read modes). One contiguous DMA per image per
tensor.

PSUM: two fixed streams, group g -> PE col-strip 32g, each with its own psum
pool (PE writes must never share banks with concurrent reads -- sharing
hard-crashes the core; distinct strips also let the PE run both groups'
matmuls concurrently). Evacuation is a lane-aligned ScalarE copy directly into
channel rows {0..3, 32..35} of u_all[36, 8, 784]; rows 4..31 are zeroed once
(broadcast DMA from a zeros input) and ride through the free-dim-bound
stats/affine ops unused.
"""

import os
import sys

import numpy as np

for _p in ("/opt/trn_rl_repo",):
    if os.path.isdir(_p) and _p not in sys.path:
        sys.path.insert(0, _p)

import concourse.bacc as bacc
import concourse.bass as bass
import concourse.tile as tile
from concourse import mybir
from concourse.bass_utils import run_bass_kernel_spmd

F32 = mybir.dt.float32
BF16 = mybir.dt.bfloat16
ALU = mybir.AluOpType
ACTF = mybir.ActivationFunctionType

N_CORES = 8
N_IMG = 8
C_IN = 32
O_TOT = 64
O_PER_CORE = O_TOT // N_CORES  # 8
N_GRP = O_PER_CORE // 4        # 2 groups of 4 channels (128 = 4*32 partitions)
HW = 28
S = HW * HW                    # 784
SH = S // 2                    # 392, per-PSUM-bank matmul width
HP, WP = HW + 2, 32            # padded image rows=30, row stride 32
PADN = HP * WP                 # 960
EPS = 1e-5
NR = 36                        # stats row span: channels at rows 0..3 & 32..35
ZN = N_IMG * S                 # 6272 zero elements for the u_all row clear

# taps handled on the Scalar engine (fused abs), per group
ACT_TAPS = {0: (1, 4), 1: (4,)}
DVE_TAPS = {g: tuple(j for j in range(9) if j not in ACT_TAPS[g]) for g in range(N_GRP)}

# f32 param blob column layout
PF_COLS = 48
PF_WT = 0        # [128, 2, 9] w, cols 0..17
PF_ONES14 = 18   # [1, 4] ones at row 0, cols 18..21
PF_NGAM = 26     # [36, 1] -gamma rows 0..3 & 32..35
PF_BETA = 27     # [36, 1] beta
PF_NWT = 28      # [128, 2, 9] -w, cols 28..45
# bf16 param blob column layout
PB_COLS = 8
PB_M2G = 0       # [128, 4] -2*G
PB_G = 4         # [128, 4] +G


def _build_nc() -> bass.Bass:
    # Bacc (not plain Bass): its compile() runs generate_event_semaphores,
    # which splits multi-wait sync info into EventSemaphore instructions --
    # walrus codegen rejects instructions with >1 sync wait otherwise.
    nc = bacc.Bacc()
    xpad_in = nc.declare_dram_parameter("xpad", [N_IMG, 128, PADN], BF16, isOutput=False)
    xodd_in = nc.declare_dram_parameter("xodd", [N_IMG, 128, PADN], BF16, isOutput=False)
    sx_in = nc.declare_dram_parameter("sxg", [N_IMG, N_GRP, S], F32, isOutput=False)
    pf_in = nc.declare_dram_parameter("pf", [128, PF_COLS], F32, isOutput=False)
    pb_in = nc.declare_dram_parameter("pb", [128, PB_COLS], BF16, isOutput=False)
    z_in = nc.declare_dram_parameter("zin", [ZN], F32, isOutput=False)
    y_out = nc.declare_dram_parameter("y", [O_PER_CORE, N_IMG, S], F32, isOutput=True)

    with tile.TileContext(nc) as tc:
        with (
            tc.tile_pool(name="singles", bufs=1) as singles,
            tc.tile_pool(name="xpb", bufs=3) as xpb_pool,
            tc.tile_pool(name="dpool", bufs=8) as d_pool,
            tc.tile_pool(name="sxp", bufs=3) as sx_pool,
            tc.tile_pool(name="psA", bufs=2, space="PSUM") as psA_pool,
            tc.tile_pool(name="psB", bufs=2, space="PSUM") as psB_pool,
            tc.tile_pool(name="small", bufs=1) as small,
        ):
            pf = singles.tile([128, PF_COLS], F32)
            pb = singles.tile([128, PB_COLS], BF16)
            nc.sync.dma_start(out=pf, in_=pf_in[:])
            nc.sync.dma_start(out=pb, in_=pb_in[:])
            wt = pf[:, PF_WT : PF_WT + 18].rearrange("p (g j) -> p g j", g=N_GRP)
            nwt = pf[:, PF_NWT : PF_NWT + 18].rearrange("p (g j) -> p g j", g=N_GRP)
            ones14 = pf[0:1, PF_ONES14 : PF_ONES14 + 4]
            ngam = pf[0:NR, PF_NGAM : PF_NGAM + 1]
            beta = pf[0:NR, PF_BETA : PF_BETA + 1]
            m2g = pb[:, PB_M2G : PB_M2G + 4]
            gsel = pb[:, PB_G : PB_G + 4]

            u_all = singles.tile([NR, N_IMG, S], F32)
            y_sb = singles.tile([NR, N_IMG, S], F32)
            stats = singles.tile([NR, N_IMG * 2, 6], F32)
            ps_pools = [psA_pool, psB_pool]
            evac_q = []   # deferred by 1 image: (img, g, psum tile)
            stats_q = []  # deferred by 2 images: img

            def emit_evac(img, g, ps):
                pos = 32 * g
                nc.scalar.copy(
                    out=u_all[pos : pos + 4, img, :].rearrange(
                        "p (h s) -> p h s", h=2
                    ),
                    in_=ps[pos : pos + 4, :, 0:SH],
                )

            def emit_stats(img):
                for h in range(2):
                    nc.vector.bn_stats(
                        out=stats[:, img * 2 + h, :],
                        in_=u_all[:, img, h * SH : (h + 1) * SH],
                    )

            for img in range(N_IMG):
                xpb = xpb_pool.tile([128, HP, WP], BF16, name="xpb", tag="xpb")
                nc.sync.dma_start(out=xpb.rearrange("p a b -> p (a b)"), in_=xpad_in[img])
                xpo = xpb_pool.tile([128, HP, WP], BF16, name="xpo", tag="xpo")
                nc.sync.dma_start(out=xpo.rearrange("p a b -> p (a b)"), in_=xodd_in[img])
                sxg = sx_pool.tile([1, N_GRP, S], F32, name="sxg", tag="sxg")
                nc.gpsimd.dma_start(out=sxg, in_=sx_in[img].rearrange("g s -> () g s"))
                if img == 0:
                    # rows 4..31 of u_all are never written by evacuation; zero
                    # them once (broadcast DMA from the zeros input) so the
                    # width-36 stats/affine ops stay finite. Emitted after the
                    # first image's loads so it doesn't delay compute start.
                    zap = z_in[:]
                    zsrc = bass.AP(
                        tensor=zap.tensor, offset=zap.offset, ap=[[0, 28], [1, ZN]]
                    )
                    nc.sync.dma_start(
                        out=u_all[4:32].rearrange("p i s -> p (i s)"), in_=zsrc
                    )

                # j-interleaved across the two groups so the PE sees
                # back-to-back matmuls on alternating col-strips (they execute
                # concurrently in the array)
                pss = [
                    ps_pools[g].tile([128, 2, 512], F32, name="ps", tag=f"ps{g}")
                    for g in range(N_GRP)
                ]
                for j in range(9):
                    dy, dx = divmod(j, 3)
                    if dx == 1:
                        src, dxx = xpo, 0
                    else:
                        src, dxx = xpb, dx
                    win = src[:, dy : dy + HW, dxx : dxx + HW]
                    for g in range(N_GRP):
                        pos = 32 * g
                        d_t = d_pool.tile([128, HW, HW], BF16, name="d_t", tag="D")
                        if j in ACT_TAPS[g]:
                            nc.scalar.activation(
                                out=d_t,
                                in_=win,
                                func=ACTF.Abs,
                                bias=nwt[:, g, j : j + 1],
                                scale=1.0,
                            )
                            lhs = gsel
                        else:
                            nc.vector.tensor_scalar(
                                out=d_t,
                                in0=win,
                                scalar1=wt[:, g, j : j + 1],
                                scalar2=None,
                                op0=ALU.min,
                            )
                            lhs = m2g
                        dm = d_t.rearrange("p a b -> p (a b)")
                        for h in range(2):
                            nc.tensor.matmul(
                                pss[g][pos : pos + 4, h, 0:SH],
                                lhs,
                                dm[:, h * SH : (h + 1) * SH],
                                start=(j == 0),
                                stop=False,
                                tile_position=(0, pos),
                            )
                for g in range(N_GRP):
                    pos = 32 * g
                    # S_x contribution (host-precomputed box filter), K=1 ones
                    for h in range(2):
                        nc.tensor.matmul(
                            pss[g][pos : pos + 4, h, 0:SH],
                            ones14,
                            sxg[0:1, g, h * SH : (h + 1) * SH],
                            start=False,
                            stop=True,
                            tile_position=(0, pos),
                        )
                # Software pipelining: the lane-aligned PSUM->SBUF evacuation
                # (ScalarE) for image i is emitted during image i+1, and
                # bn_stats (DVE) for image i during image i+2 — so neither
                # engine's in-order stream stalls waiting for image i's
                # matmuls before starting image i+1's taps.
                evac_q.append((img, pss))
                if len(evac_q) > 1:
                    eimg, eps = evac_q.pop(0)
                    for g in range(N_GRP):
                        emit_evac(eimg, g, eps[g])
                    stats_q.append(eimg)
                if len(stats_q) > 1:
                    emit_stats(stats_q.pop(0))

            eimg, eps = evac_q.pop(0)
            for g in range(N_GRP):
                emit_evac(eimg, g, eps[g])
            stats_q.append(eimg)
            for eimg in stats_q:
                emit_stats(eimg)

            mv = small.tile([NR, 2], F32)
            nc.vector.bn_aggr(out=mv, in_=stats)
            eps_sb = small.tile([NR, 1], F32)
            nc.vector.memset(eps_sb, EPS)
            stdv = small.tile([NR, 1], F32)
            nc.scalar.activation(out=stdv, in_=mv[:, 1:2], func=ACTF.Sqrt, bias=eps_sb)
            rinv = small.tile([NR, 1], F32)
            nc.vector.reciprocal(out=rinv, in_=stdv)
            a_t = small.tile([NR, 1], F32)
            nc.vector.tensor_tensor(out=a_t, in0=rinv, in1=ngam, op=ALU.mult)
            t2 = small.tile([NR, 1], F32)
            nc.vector.tensor_tensor(out=t2, in0=a_t, in1=mv[:, 0:1], op=ALU.mult)
            b_t = small.tile([NR, 1], F32)
            nc.vector.tensor_tensor(out=b_t, in0=beta, in1=t2, op=ALU.subtract)

            # y = u*A + B, split across DVE and ACT (free-dim bound; the unused
            # rows 4..31 ride along for free)
            nc.vector.tensor_scalar(
                out=y_sb[:, 0:4, :],
                in0=u_all[:, 0:4, :],
                scalar1=a_t,
                scalar2=b_t,
                op0=ALU.mult,
                op1=ALU.add,
            )
            nc.scalar.activation(
                out=y_sb[:, 4:8, :],
                in_=u_all[:, 4:8, :],
                func=ACTF.Identity,
                bias=b_t,
                scale=a_t,
            )
            for g in range(N_GRP):
                for e in range(2):
                    nc.sync.dma_start(
                        out=y_out[4 * g : 4 * g + 4, 4 * e : 4 * e + 4, :],
                        in_=y_sb[32 * g : 32 * g + 4, 4 * e : 4 * e + 4, :],
                    )
    nc.finalize()
    return nc


_NC_CACHE: dict = {}


def _get_nc() -> bass.Bass:
    if "nc" not in _NC_CACHE:
        _NC_CACHE["nc"] = _build_nc()
    return _NC_CACHE["nc"]


_GSEL = np.kron(np.eye(4, dtype=np.float32), np.ones((32, 1), dtype=np.float32))


def _bf16(a):
    import ml_dtypes

    return np.ascontiguousarray(a).astype(ml_dtypes.bfloat16)


def _prep_x(x):
    """[8, 32, 28, 28] f32 -> (xpad bf16 [8,128,960], xodd bf16, sxg f32 [8,2,784]).

    xpad: zero-padded to 30x32 (row stride 32), replicated into 4 partition
    blocks, bf16. xodd: same shifted left one element (dx=1 alignment).
    sxg[n,g]: sum over channels and over this group's min-trick taps of the
    shifted (bf16-rounded, matching the device data) input windows.
    """
    xp = np.zeros((N_IMG, C_IN, HP, WP), dtype=np.float32)
    xp[:, :, 1 : 1 + HW, 1 : 1 + HW] = x
    xb1 = _bf16(xp)  # [8, 32, 30, 32]
    xb = np.tile(xb1.reshape(N_IMG, C_IN, PADN), (1, 4, 1))
    xo = np.zeros_like(xb)
    xo[:, :, : PADN - 1] = xb[:, :, 1:]

    csum = xb1.astype(np.float32).sum(axis=1)  # [8, 30, 32]
    sxg = np.zeros((N_IMG, N_GRP, HW, HW), dtype=np.float32)
    for g in range(N_GRP):
        for j in DVE_TAPS[g]:
            dy, dx = divmod(j, 3)
            sxg[:, g] += csum[:, dy : dy + HW, dx : dx + HW]
    return xb, xo, np.ascontiguousarray(sxg.reshape(N_IMG, N_GRP, S))


def _in_maps(x, W, gamma, beta):
    x = np.ascontiguousarray(x, dtype=np.float32)
    W = np.asarray(W, dtype=np.float32)
    gamma = np.asarray(gamma, dtype=np.float32)
    beta = np.asarray(beta, dtype=np.float32)
    xb, xo, sxg = _prep_x(x)
    pb = np.zeros((128, PB_COLS), dtype=np.float32)
    pb[:, PB_M2G : PB_M2G + 4] = -2.0 * _GSEL
    pb[:, PB_G : PB_G + 4] = _GSEL
    pb = _bf16(pb)
    zin = np.zeros((ZN,), dtype=np.float32)
    maps = []
    for core in range(N_CORES):
        base = core * O_PER_CORE
        w8 = W[base : base + O_PER_CORE].reshape(N_GRP, 4, C_IN, 9)
        wt = w8.transpose(1, 2, 0, 3).reshape(128, N_GRP * 9)
        pf = np.zeros((128, PF_COLS), dtype=np.float32)
        pf[:, PF_WT : PF_WT + 18] = wt
        pf[:, PF_NWT : PF_NWT + 18] = -wt
        pf[0, PF_ONES14 : PF_ONES14 + 4] = 1.0
        gam = gamma[base : base + O_PER_CORE]
        bet = beta[base : base + O_PER_CORE]
        pf[0:4, PF_NGAM] = -gam[0:4]
        pf[32:36, PF_NGAM] = -gam[4:8]
        pf[0:4, PF_BETA] = bet[0:4]
        pf[32:36, PF_BETA] = bet[4:8]
        maps.append(
            {"xpad": xb, "xodd": xo, "sxg": sxg, "pf": pf, "pb": pb, "zin": zin}
        )
    return maps


def _gather(results) -> np.ndarray:
    y = np.empty((N_IMG, O_TOT, HW, HW), dtype=np.float32)
    for core in range(N_CORES):
        yo = results[core]["y"]  # [o_local, img, s]
        y[:, core * O_PER_CORE : (core + 1) * O_PER_CORE] = yo.transpose(
            1, 0, 2
        ).reshape(N_IMG, O_PER_CORE, HW, HW)
    return y


def run(x, W, gamma, beta, trace=False, **trace_kwargs):
    nc = _get_nc()
    maps = _in_maps(x, W, gamma, beta)
    res = run_bass_kernel_spmd(
        nc, maps, list(range(N_CORES)), trace=trace, **trace_kwargs
    )
    return _gather(res.results), res


def kernel(x, W, gamma, beta) -> np.ndarray:
    y, _ = run(x, W, gamma, beta)
    return y



# revision 3
# speedup vs baseline: 1.4378x; 1.4378x over previous
"""AdderConv (AdderNet conv 3x3 + BatchNorm2d, training stats) on 8 trn2 cores.

Reference computation:
  u[n,o,yx] = sum_{c,dy,dx} |x[n,c,y+dy-1,x+dx-1] - W[o,c,dy,dx]|   (zero padded)
  out = -u, then BatchNorm2d over (n, y, x) per channel o with affine gamma/beta.

Sharding: output channels. Core k owns channels [8k, 8k+8); every core reads the
full x. BatchNorm stats are per-channel, hence fully core-local (no collectives).

Key algebra: |x - w| = x + w - 2*min(x, w).
  u[o,s] = S_x(s) + S_w(o) - 2 * sum_k min(x_k(s), w_ok)
  - S_w(o) is constant per channel -> shift-invariant under BatchNorm -> dropped.
  - S_x(s) = sum_{c,j in min-taps} x[c, s+d_j] is channel-independent: a 3x3
    box filter of the channel-summed input, precomputed host-side as a bf16
    hi/lo pair (hi + residual, exact to ~2^-17) and accumulated into PSUM by a
    K=2 ones matmul at bf16 matmul rate (1 cycle/row; an f32 rhs would cost 4x).
  - min(x, w) is ONE stock DVE tensor_scalar(op0=min) per tap: bf16 4x mode.
  - Taps j in {1, 4} (both groups) run on the Scalar engine as fused |x + (-w)|
    via activation(func=Abs, bias=-w), excluded from S_x, balancing DVE vs ACT.

PE layout: both 4-channel groups share ONE 8-partition PSUM strip (cols 0..7 of
the PE array). Group g's matmuls use a [128, 8] lhs whose 4 non-zero columns
are 4g..4g+3; the other group's columns accumulate zeros. This gives: a single
accumulation stream per bank, ONE [8, 784] evacuation per image, a single K=2
sxg matmul per half, and channels 0..7 landing on adjacent partitions (no
zero-padded stats rows). In the timeline cost model the PE charges per moving
row regardless of column count, so merging strips costs nothing.

P-state: the Tensor engine ramps (0.65 -> 1.2 -> 2.4 GHz) with ~7.5us of
continuous busy needed to hit full clock, and instruction cost is latched at
decode. A burst of 64-row junk matmuls at kernel start (during the input DMA
dead time) starts the ramp early so real matmuls run at full speed; the
stream is ordered so the PE never goes idle mid-kernel.

Data staging: x is pre-padded/replicated/bf16-cast on the host into
xx[8, 128, 2*960] (partitions = 4 o-slots x 32 channels; halves = normal and
one-element-shifted copies, keeping dx=1 windows 4-byte aligned for the DVE
packed read modes) -- ONE contiguous DMA per image.

Tail: bn_stats per image is deferred by one image; after the last image's
evacuation, bn_aggr + a fused Rsqrt (same ACT table set as Abs/Copy/Identity,
so no mid-kernel table loads) produce the affine constants, and y = u*A + B is
split DVE/ACT/Pool (images 0-2 / 3-5 / 6-7) with one output DMA per chunk.
"""

import os
import sys

import numpy as np

for _p in ("/opt/trn_rl_repo",):
    if os.path.isdir(_p) and _p not in sys.path:
        sys.path.insert(0, _p)

import concourse.bacc as bacc
import concourse.bass as bass
import concourse.tile as tile
from concourse import mybir
from concourse.bass_utils import run_bass_kernel_spmd

F32 = mybir.dt.float32
BF16 = mybir.dt.bfloat16
ALU = mybir.AluOpType
ACTF = mybir.ActivationFunctionType

N_CORES = 8
N_IMG = 8
C_IN = 32
O_TOT = 64
O_PER_CORE = O_TOT // N_CORES  # 8
N_GRP = 2                      # 2 groups of 4 channels (128 = 4*32 partitions)
HW = 28
S = HW * HW                    # 784
SH = S // 2                    # 392, per-PSUM-bank matmul width
HP, WP = HW + 2, 32            # padded image rows=30, row stride 32
PADN = HP * WP                 # 960
EPS = 1e-5
NWARM = 56                     # 64-row junk matmuls to ramp the PE p-state

ACT_TAPS = (1, 4)              # scalar-engine taps (same for both groups)
DVE_TAPS = tuple(j for j in range(9) if j not in ACT_TAPS)

# f32 param blob column layout
PF_COLS = 40
PF_WT = 0        # [128, 2, 9] w, cols 0..17
PF_NWT = 18      # [128, 2, 9] -w, cols 18..35
PF_NGAM = 36     # [8, 1] -gamma
PF_BETA = 37     # [8, 1] beta
# bf16 param blob column layout
PB_COLS = 40
PB_M2G = 0       # [128, 8] -2*G per group at 0..7 / 8..15
PB_G = 16        # [128, 8] +G per group at 16..23 / 24..31
PB_ONES = 32     # [2, 8] ones at rows 0..1, cols 32..39


def _build_nc() -> bass.Bass:
    # Bacc (not plain Bass): its compile() runs generate_event_semaphores,
    # which splits multi-wait sync info into EventSemaphore instructions --
    # walrus codegen rejects instructions with >1 sync wait otherwise.
    nc = bacc.Bacc()
    xx_in = nc.declare_dram_parameter("xx", [N_IMG, 128, 2 * PADN], BF16, isOutput=False)
    sx_in = nc.declare_dram_parameter("sxg", [N_IMG, 2, S], BF16, isOutput=False)
    pf_in = nc.declare_dram_parameter("pf", [128, PF_COLS], F32, isOutput=False)
    pb_in = nc.declare_dram_parameter("pb", [128, PB_COLS], BF16, isOutput=False)
    y_out = nc.declare_dram_parameter("y", [O_PER_CORE, N_IMG, S], F32, isOutput=True)

    with tile.TileContext(nc) as tc:
        with (
            tc.tile_pool(name="singles", bufs=1) as singles,
            tc.tile_pool(name="xxp", bufs=3) as xx_pool,
            tc.tile_pool(name="dpool", bufs=12) as d_pool,
            tc.tile_pool(name="sxp", bufs=3) as sx_pool,
            tc.tile_pool(name="ps", bufs=3, space="PSUM") as ps_pool,
            tc.tile_pool(name="wps", bufs=1, space="PSUM") as wps_pool,
            tc.tile_pool(name="small", bufs=1) as small,
        ):
            junk = singles.tile([128, 64], BF16)
            nc.vector.memset(junk, 0.5)
            eps_sb = small.tile([O_PER_CORE, 1], F32)
            nc.vector.memset(eps_sb, EPS)
            pf = singles.tile([128, PF_COLS], F32)
            pb = singles.tile([128, PB_COLS], BF16)
            nc.sync.dma_start(out=pf, in_=pf_in[:])
            nc.sync.dma_start(out=pb, in_=pb_in[:])
            wt = pf[:, PF_WT : PF_WT + 18].rearrange("p (g j) -> p g j", g=N_GRP)
            nwt = pf[:, PF_NWT : PF_NWT + 18].rearrange("p (g j) -> p g j", g=N_GRP)
            ngam = pf[0:O_PER_CORE, PF_NGAM : PF_NGAM + 1]
            beta = pf[0:O_PER_CORE, PF_BETA : PF_BETA + 1]
            m2g = [pb[:, PB_M2G + 8 * g : PB_M2G + 8 * g + 8] for g in range(N_GRP)]
            gsel = [pb[:, PB_G + 8 * g : PB_G + 8 * g + 8] for g in range(N_GRP)]
            ones8 = pb[0:2, PB_ONES : PB_ONES + 8]

            u_all = singles.tile([O_PER_CORE, N_IMG, S], F32)
            y_sb = singles.tile([O_PER_CORE, N_IMG, S], F32)
            stats = singles.tile([O_PER_CORE, N_IMG * 2, 6], F32)

            # Preload the ACT function table (Abs/Copy/Identity/Rsqrt share
            # one set) during the input DMA dead time.
            tjunk = small.tile([8, 1], F32)
            nc.scalar.activation(out=tjunk, in_=eps_sb, func=ACTF.Abs, scale=1.0)

            # PE p-state warmup: junk matmuls into a scratch PSUM bank.
            wps = wps_pool.tile([128, 512], F32)
            for _ in range(NWARM):
                nc.tensor.matmul(
                    wps[0:8, 0:64], junk[:, 0:8], junk[:, 0:64],
                    start=True, stop=True, tile_position=(0, 0),
                )

            evac_q = []   # deferred by 1 image: (img, psum tile)
            stats_q = []  # deferred until evac done: img

            def emit_evac(img, ps):
                nc.scalar.copy(
                    out=u_all[0:O_PER_CORE, img, :].rearrange(
                        "p (h s) -> p h s", h=2
                    ),
                    in_=ps[0:O_PER_CORE, :, 0:SH],
                )

            def emit_stats(img):
                for h in range(2):
                    nc.vector.bn_stats(
                        out=stats[:, img * 2 + h, :],
                        in_=u_all[:, img, h * SH : (h + 1) * SH],
                    )

            for img in range(N_IMG):
                xxt = xx_pool.tile([128, 2, HP, WP], BF16, name="xx", tag="xx")
                nc.sync.dma_start(
                    out=xxt.rearrange("p a b c -> p (a b c)"), in_=xx_in[img]
                )
                sxg = sx_pool.tile([2, S], BF16, name="sxg", tag="sxg")
                nc.gpsimd.dma_start(out=sxg, in_=sx_in[img])
                ps = ps_pool.tile([128, 2, 512], F32, name="ps", tag="ps")

                # PSUM->SBUF evacuation of the previous image runs first on
                # ACT so bn_stats (DVE, deferred one image) never stalls.
                if evac_q:
                    eimg, eps_t = evac_q.pop(0)
                    emit_evac(eimg, eps_t)
                    stats_q.append(eimg)

                def win(j):
                    dy, dx = divmod(j, 3)
                    half, dxx = (1, 0) if dx == 1 else (0, dx)
                    return xxt[:, half, dy : dy + HW, dxx : dxx + HW]

                first = True
                for g in range(N_GRP):
                    for j in DVE_TAPS:
                        d_t = d_pool.tile([128, HW, HW], BF16, name="d_t", tag="D")
                        nc.vector.tensor_scalar(
                            out=d_t, in0=win(j),
                            scalar1=wt[:, g, j : j + 1], scalar2=None,
                            op0=ALU.min,
                        )
                        dm = d_t.rearrange("p a b -> p (a b)")
                        for h in range(2):
                            nc.tensor.matmul(
                                ps[0:8, h, 0:SH], m2g[g],
                                dm[:, h * SH : (h + 1) * SH],
                                start=first, stop=False, tile_position=(0, 0),
                            )
                        first = False
                for g in range(N_GRP):
                    for j in ACT_TAPS:
                        d_t = d_pool.tile([128, HW, HW], BF16, name="d_a", tag="D")
                        nc.scalar.activation(
                            out=d_t, in_=win(j), func=ACTF.Abs,
                            bias=nwt[:, g, j : j + 1], scale=1.0,
                        )
                        dm = d_t.rearrange("p a b -> p (a b)")
                        for h in range(2):
                            nc.tensor.matmul(
                                ps[0:8, h, 0:SH], gsel[g],
                                dm[:, h * SH : (h + 1) * SH],
                                start=False, stop=False, tile_position=(0, 0),
                            )
                # S_x contribution: K=2 ones matmul over the bf16 hi/lo pair.
                for h in range(2):
                    nc.tensor.matmul(
                        ps[0:8, h, 0:SH], ones8,
                        sxg[0:2, h * SH : (h + 1) * SH],
                        start=False, stop=True, tile_position=(0, 0),
                    )
                evac_q.append((img, ps))
                if stats_q:
                    emit_stats(stats_q.pop(0))

            eimg, eps_t = evac_q.pop(0)
            emit_evac(eimg, eps_t)
            stats_q.append(eimg)
            for eimg in stats_q:
                emit_stats(eimg)

            mv = small.tile([O_PER_CORE, 2], F32)
            nc.vector.bn_aggr(out=mv, in_=stats)
            stdv = small.tile([O_PER_CORE, 1], F32)
            nc.scalar.activation(
                out=stdv, in_=mv[:, 1:2], func=ACTF.Sqrt, bias=eps_sb, scale=1.0
            )
            rinv = small.tile([O_PER_CORE, 1], F32)
            nc.vector.reciprocal(out=rinv, in_=stdv)
            a_t = small.tile([O_PER_CORE, 1], F32)
            nc.vector.tensor_tensor(out=a_t, in0=rinv, in1=ngam, op=ALU.mult)
            t2 = small.tile([O_PER_CORE, 1], F32)
            nc.vector.tensor_tensor(out=t2, in0=a_t, in1=mv[:, 0:1], op=ALU.mult)
            b_t = small.tile([O_PER_CORE, 1], F32)
            nc.vector.tensor_tensor(out=b_t, in0=beta, in1=t2, op=ALU.subtract)

            # y = u*A + B, split DVE / ACT / Pool; one output DMA per chunk.
            nc.vector.tensor_scalar(
                out=y_sb[:, 0:3, :], in0=u_all[:, 0:3, :],
                scalar1=a_t, scalar2=b_t, op0=ALU.mult, op1=ALU.add,
            )
            nc.sync.dma_start(out=y_out[:, 0:3, :], in_=y_sb[:, 0:3, :])
            nc.scalar.activation(
                out=y_sb[:, 3:6, :], in_=u_all[:, 3:6, :],
                func=ACTF.Identity, bias=b_t, scale=a_t,
            )
            nc.sync.dma_start(out=y_out[:, 3:6, :], in_=y_sb[:, 3:6, :])
            nc.gpsimd.tensor_scalar(
                out=y_sb[:, 6:8, :], in0=u_all[:, 6:8, :],
                scalar1=a_t, scalar2=b_t, op0=ALU.mult, op1=ALU.add,
            )
            nc.sync.dma_start(out=y_out[:, 6:8, :], in_=y_sb[:, 6:8, :])
    nc.finalize()
    return nc


_NC_CACHE: dict = {}


def _get_nc() -> bass.Bass:
    if "nc" not in _NC_CACHE:
        _NC_CACHE["nc"] = _build_nc()
    return _NC_CACHE["nc"]


def _bf16(a):
    import ml_dtypes

    return np.ascontiguousarray(a).astype(ml_dtypes.bfloat16)


def _prep_x(x):
    """[8, 32, 28, 28] f32 -> (xx bf16 [8,128,1920], sxg bf16 [8,2,784]).

    xx: zero-padded to 30x32 (row stride 32), replicated into 4 partition
    blocks, bf16; first 960 columns normal, last 960 shifted left one element
    (dx=1 alignment). sxg: bf16 hi/lo split of the channel-and-tap-summed
    input windows over the min-trick taps.
    """
    xp = np.zeros((N_IMG, C_IN, HP, WP), dtype=np.float32)
    xp[:, :, 1 : 1 + HW, 1 : 1 + HW] = x
    xb1 = _bf16(xp)  # [8, 32, 30, 32]
    xb = np.tile(xb1.reshape(N_IMG, C_IN, PADN), (1, 4, 1))
    xo = np.zeros_like(xb)
    xo[:, :, : PADN - 1] = xb[:, :, 1:]
    xx = np.concatenate([xb[:, :, None, :], xo[:, :, None, :]], axis=2)
    xx = np.ascontiguousarray(xx.reshape(N_IMG, 128, 2 * PADN))

    csum = xb1.astype(np.float32).sum(axis=1)  # [8, 30, 32]
    sx = np.zeros((N_IMG, HW, HW), dtype=np.float32)
    for j in DVE_TAPS:
        dy, dx = divmod(j, 3)
        sx += csum[:, dy : dy + HW, dx : dx + HW]
    sx = sx.reshape(N_IMG, S)
    hi = _bf16(sx)
    lo = _bf16(sx - hi.astype(np.float32))
    sxg = np.ascontiguousarray(np.stack([hi, lo], axis=1))
    return xx, sxg


def _in_maps(x, W, gamma, beta):
    x = np.ascontiguousarray(x, dtype=np.float32)
    W = np.asarray(W, dtype=np.float32)
    gamma = np.asarray(gamma, dtype=np.float32)
    beta = np.asarray(beta, dtype=np.float32)
    xx, sxg = _prep_x(x)

    slot = np.arange(128) // 32  # partition -> o-slot
    gmat = (slot[:, None] == np.arange(4)[None, :]).astype(np.float32)
    pb = np.zeros((128, PB_COLS), dtype=np.float32)
    for g in range(N_GRP):
        pb[:, PB_M2G + 8 * g + 4 * g : PB_M2G + 8 * g + 4 * g + 4] = -2.0 * gmat
        pb[:, PB_G + 8 * g + 4 * g : PB_G + 8 * g + 4 * g + 4] = gmat
    pb[0:2, PB_ONES : PB_ONES + 8] = 1.0
    pb = _bf16(pb)

    maps = []
    for core in range(N_CORES):
        base = core * O_PER_CORE
        w8 = W[base : base + O_PER_CORE].reshape(N_GRP, 4, C_IN, 9)
        wt = w8.transpose(1, 2, 0, 3).reshape(128, N_GRP * 9)
        pf = np.zeros((128, PF_COLS), dtype=np.float32)
        pf[:, PF_WT : PF_WT + 18] = wt
        pf[:, PF_NWT : PF_NWT + 18] = -wt
        pf[0:O_PER_CORE, PF_NGAM] = -gamma[base : base + O_PER_CORE]
        pf[0:O_PER_CORE, PF_BETA] = beta[base : base + O_PER_CORE]
        maps.append({"xx": xx, "sxg": sxg, "pf": pf, "pb": pb})
    return maps


def _gather(results) -> np.ndarray:
    y = np.empty((N_IMG, O_TOT, HW, HW), dtype=np.float32)
    for core in range(N_CORES):
        yo = results[core]["y"]  # [o_local, img, s]
        y[:, core * O_PER_CORE : (core + 1) * O_PER_CORE] = yo.transpose(
            1, 0, 2
        ).reshape(N_IMG, O_PER_CORE, HW, HW)
    return y


def run(x, W, gamma, beta, trace=False, **trace_kwargs):
    nc = _get_nc()
    maps = _in_maps(x, W, gamma, beta)
    res = run_bass_kernel_spmd(
        nc, maps, list(range(N_CORES)), trace=trace, **trace_kwargs
    )
    return _gather(res.results), res


def kernel(x, W, gamma, beta) -> np.ndarray:
    y, _ = run(x, W, gamma, beta)
    return y


# revision 13
# speedup vs baseline: 1.5465x; 1.0756x over previous
"""AdderConv (AdderNet conv 3x3 + BatchNorm2d, training stats) on 8 trn2 cores.

Reference computation:
  u[n,o,yx] = sum_{c,dy,dx} |x[n,c,y+dy-1,x+dx-1] - W[o,c,dy,dx]|   (zero padded)
  out = -u, then BatchNorm2d over (n, y, x) per channel o with affine gamma/beta.

Sharding: output channels. Core k owns channels [8k, 8k+8); every core reads the
full x. BatchNorm stats are per-channel, hence fully core-local (no collectives).

Key algebra: |x - w| = x + w - 2*min(x, w).
  u[o,s] = S_x(s) + S_w(o) - 2 * sum_k min(x_k(s), w_ok)
  - S_w(o) is constant per channel -> shift-invariant under BatchNorm -> dropped.
  - S_x(s) = sum_{c,j in min-taps} x[c, s+d_j] is channel-independent: a 3x3
    box filter of the channel-summed input, precomputed host-side as a bf16
    hi/lo pair (hi + residual, exact to ~2^-17) and accumulated into PSUM by a
    K=2 ones matmul at bf16 matmul rate (1 cycle/row; an f32 rhs would cost 4x).
  - min(x, w) is ONE stock DVE tensor_scalar(op0=min) per tap: bf16 4x mode.
  - Taps j in {1, 4} (both groups) run on the Scalar engine as fused |x + (-w)|
    via activation(func=Abs, bias=-w), excluded from S_x, balancing DVE vs ACT.

PE layout: both 4-channel groups share ONE 8-partition PSUM strip (cols 0..7 of
the PE array). Group g's matmuls use a [128, 8] lhs whose 4 non-zero columns
are 4g..4g+3; the other group's columns accumulate zeros. This gives: a single
accumulation stream per bank, ONE [8, 784] evacuation per image, a single K=2
sxg matmul per half, and channels 0..7 landing on adjacent partitions (no
zero-padded stats rows). In the timeline cost model the PE charges per moving
row regardless of column count, so merging strips costs nothing.

P-state: the Tensor engine ramps (0.65 -> 1.2 -> 2.4 GHz) with ~7.5us of
continuous busy needed to hit full clock, and instruction cost is latched at
decode. A burst of 64-row junk matmuls at kernel start (during the input DMA
dead time) starts the ramp early so real matmuls run at full speed; the
stream is ordered so the PE never goes idle mid-kernel.

Data staging: x is pre-padded/replicated/bf16-cast on the host into
xx[8, 128, 2*960] (partitions = 4 o-slots x 32 channels; halves = normal and
one-element-shifted copies, keeping dx=1 windows 4-byte aligned for the DVE
packed read modes) -- ONE contiguous DMA per image.

Tail: bn_stats per image is deferred by one image; after the last image's
evacuation, bn_aggr + a fused Rsqrt (same ACT table set as Abs/Copy/Identity,
so no mid-kernel table loads) produce the affine constants, and y = u*A + B is
split DVE/ACT/Pool (images 0-2 / 3-5 / 6-7) with one output DMA per chunk.
"""

import os
import sys

import numpy as np

for _p in ("/opt/trn_rl_repo",):
    if os.path.isdir(_p) and _p not in sys.path:
        sys.path.insert(0, _p)

import concourse.bacc as bacc
import concourse.bass as bass
import concourse.tile as tile
from concourse import mybir
from concourse.bass_utils import run_bass_kernel_spmd

F32 = mybir.dt.float32
BF16 = mybir.dt.bfloat16
ALU = mybir.AluOpType
ACTF = mybir.ActivationFunctionType

N_CORES = 8
N_IMG = 8
C_IN = 32
O_TOT = 64
O_PER_CORE = O_TOT // N_CORES  # 8
N_GRP = 2                      # 2 groups of 4 channels (128 = 4*32 partitions)
HW = 28
S = HW * HW                    # 784
SH = S // 2                    # 392, per-PSUM-bank matmul width
HP, WP = HW + 2, 32            # padded image rows=30, row stride 32
PADN = HP * WP                 # 960
EPS = 1e-5
NWARM = 62                     # 64-row junk matmuls to ramp the PE p-state

ACT_TAPS = (1, 4)              # scalar-engine taps (same for both groups)
DVE_TAPS = tuple(j for j in range(9) if j not in ACT_TAPS)

# f32 param blob column layout
PF_COLS = 40
PF_WT = 0        # [128, 2, 9] w, cols 0..17
PF_NWT = 18      # [128, 2, 9] -w, cols 18..35
PF_NGAM = 36     # [8, 1] -gamma
PF_BETA = 37     # [8, 1] beta
# bf16 param blob column layout
PB_COLS = 40
PB_M2G = 0       # [128, 8] -2*G per group at 0..7 / 8..15
PB_G = 16        # [128, 8] +G per group at 16..23 / 24..31
PB_ONES = 32     # [2, 8] ones at rows 0..1, cols 32..39


def _build_nc() -> bass.Bass:
    # Bacc (not plain Bass): its compile() runs generate_event_semaphores,
    # which splits multi-wait sync info into EventSemaphore instructions --
    # walrus codegen rejects instructions with >1 sync wait otherwise.
    nc = bacc.Bacc()
    xx_in = nc.declare_dram_parameter("xx", [N_IMG, 128, 2 * PADN], BF16, isOutput=False)
    sx_in = nc.declare_dram_parameter("sxg", [N_IMG, 2, S], BF16, isOutput=False)
    pf_in = nc.declare_dram_parameter("pf", [128, PF_COLS], F32, isOutput=False)
    pb_in = nc.declare_dram_parameter("pb", [128, PB_COLS], BF16, isOutput=False)
    y_out = nc.declare_dram_parameter("y", [O_PER_CORE, N_IMG, S], F32, isOutput=True)

    with tile.TileContext(nc) as tc:
        with (
            tc.tile_pool(name="singles", bufs=1) as singles,
            tc.tile_pool(name="xxp", bufs=3) as xx_pool,
            tc.tile_pool(name="dpool", bufs=20) as d_pool,
            tc.tile_pool(name="sxp", bufs=3) as sx_pool,
            tc.tile_pool(name="ps", bufs=3, space="PSUM") as ps_pool,
            tc.tile_pool(name="wps", bufs=1, space="PSUM") as wps_pool,
            tc.tile_pool(name="small", bufs=1) as small,
        ):
            junk = singles.tile([128, 64], BF16)
            nc.vector.memset(junk, 0.5)
            eps_sb = small.tile([O_PER_CORE, 1], F32)
            nc.vector.memset(eps_sb, EPS)
            # Params go through the Pool SWDGE queue so the SP/HWDGE queue is
            # free to start streaming image 0 immediately.
            pf = singles.tile([128, PF_COLS], F32)
            pb = singles.tile([128, PB_COLS], BF16)
            nc.gpsimd.dma_start(out=pf, in_=pf_in[:])
            nc.gpsimd.dma_start(out=pb, in_=pb_in[:])
            wt = pf[:, PF_WT : PF_WT + 18].rearrange("p (g j) -> p g j", g=N_GRP)
            nwt = pf[:, PF_NWT : PF_NWT + 18].rearrange("p (g j) -> p g j", g=N_GRP)
            ngam = pf[0:O_PER_CORE, PF_NGAM : PF_NGAM + 1]
            beta = pf[0:O_PER_CORE, PF_BETA : PF_BETA + 1]
            m2g = [pb[:, PB_M2G + 8 * g : PB_M2G + 8 * g + 8] for g in range(N_GRP)]
            gsel = [pb[:, PB_G + 8 * g : PB_G + 8 * g + 8] for g in range(N_GRP)]
            ones8 = pb[0:2, PB_ONES : PB_ONES + 8]

            u_all = singles.tile([O_PER_CORE, N_IMG, S], F32)
            y_sb = singles.tile([O_PER_CORE, N_IMG, S], F32)
            stats = singles.tile([O_PER_CORE, N_IMG * 2, 6], F32)

            # Preload the ACT function table during the input DMA dead time.
            # Sqrt-then-Abs pins the one set holding Sqrt/Abs/Copy/Identity
            # (sqrt_and_others), so no mid-kernel or tail table swaps occur.
            tjunk = small.tile([8, 1], F32)
            nc.scalar.activation(out=tjunk, in_=eps_sb, func=ACTF.Sqrt, scale=1.0)
            nc.scalar.activation(out=tjunk, in_=eps_sb, func=ACTF.Abs, scale=1.0)

            # PE p-state warmup: junk matmuls into a scratch PSUM bank.
            wps = wps_pool.tile([128, 512], F32)
            for _ in range(NWARM):
                nc.tensor.matmul(
                    wps[0:8, 0:64], junk[:, 0:8], junk[:, 0:64],
                    start=True, stop=True, tile_position=(0, 0),
                )

            evac_q = []   # deferred by 1 image: (img, psum tile)
            stats_q = []  # deferred until evac done: img

            def emit_evac(img, ps):
                nc.scalar.copy(
                    out=u_all[0:O_PER_CORE, img, :].rearrange(
                        "p (h s) -> p h s", h=2
                    ),
                    in_=ps[0:O_PER_CORE, :, 0:SH],
                )

            def emit_stats(img):
                for h in range(2):
                    nc.vector.bn_stats(
                        out=stats[:, img * 2 + h, :],
                        in_=u_all[:, img, h * SH : (h + 1) * SH],
                    )

            for img in range(N_IMG):
                xxt = xx_pool.tile([128, 2, HP, WP], BF16, name="xx", tag="xx")
                xf = xxt.rearrange("p a b c -> p (a b c)")
                if img == 0:
                    # Split the first image's load so the second half's HWDGE
                    # descriptor generation overlaps the first's transfer.
                    nc.sync.dma_start(out=xf[:, 0:PADN], in_=xx_in[img][:, 0:PADN])
                    nc.sync.dma_start(
                        out=xf[:, PADN : 2 * PADN],
                        in_=xx_in[img][:, PADN : 2 * PADN],
                    )
                else:
                    nc.sync.dma_start(out=xf, in_=xx_in[img])
                sxg = sx_pool.tile([2, S], BF16, name="sxg", tag="sxg")
                nc.gpsimd.dma_start(out=sxg, in_=sx_in[img])
                ps = ps_pool.tile([128, 2, 512], F32, name="ps", tag="ps")

                last = img == N_IMG - 1
                # PSUM->SBUF evacuation of the previous image runs first on
                # ACT so bn_stats (DVE, deferred one image) never stalls.
                if evac_q:
                    eimg, eps_t = evac_q.pop(0)
                    emit_evac(eimg, eps_t)
                    stats_q.append(eimg)

                def win(j):
                    dy, dx = divmod(j, 3)
                    half, dxx = (1, 0) if dx == 1 else (0, dx)
                    return xxt[:, half, dy : dy + HW, dxx : dxx + HW]

                # The last image runs h-major (all bank-0 matmuls, then all
                # bank-1) so its first half evacuates + runs bn_stats while
                # the PE is still busy with the second half, shrinking the
                # serial tail.
                dms = {}
                h_groups = [(0, 1)] if not last else [(0,), (1,)]
                for hg in h_groups:
                    first = True
                    for g in range(N_GRP):
                        for j in DVE_TAPS:
                            if (g, j) not in dms:
                                d_t = d_pool.tile(
                                    [128, HW, HW], BF16, name="d_t", tag="D"
                                )
                                nc.vector.tensor_scalar(
                                    out=d_t, in0=win(j),
                                    scalar1=wt[:, g, j : j + 1], scalar2=None,
                                    op0=ALU.min,
                                )
                                dms[(g, j)] = d_t.rearrange("p a b -> p (a b)")
                            dm = dms[(g, j)]
                            for h in hg:
                                nc.tensor.matmul(
                                    ps[0:8, h, 0:SH], m2g[g],
                                    dm[:, h * SH : (h + 1) * SH],
                                    start=first, stop=False,
                                    tile_position=(0, 0),
                                )
                            first = False
                    for g in range(N_GRP):
                        for j in ACT_TAPS:
                            if (g, j) not in dms:
                                d_t = d_pool.tile(
                                    [128, HW, HW], BF16, name="d_a", tag="D"
                                )
                                nc.scalar.activation(
                                    out=d_t, in_=win(j), func=ACTF.Abs,
                                    bias=nwt[:, g, j : j + 1], scale=1.0,
                                )
                                dms[(g, j)] = d_t.rearrange("p a b -> p (a b)")
                            dm = dms[(g, j)]
                            for h in hg:
                                nc.tensor.matmul(
                                    ps[0:8, h, 0:SH], gsel[g],
                                    dm[:, h * SH : (h + 1) * SH],
                                    start=False, stop=False,
                                    tile_position=(0, 0),
                                )
                    # S_x contribution: K=2 ones matmul over the bf16 hi/lo
                    # pair (bf16 rhs: 1 cycle/row; f32 would cost 4x).
                    for h in hg:
                        nc.tensor.matmul(
                            ps[0:8, h, 0:SH], ones8,
                            sxg[0:2, h * SH : (h + 1) * SH],
                            start=False, stop=True, tile_position=(0, 0),
                        )
                    if last:
                        # Flush the deferred previous-image stats after this
                        # image's min-taps, then per-half evacuation + stats.
                        if stats_q:
                            emit_stats(stats_q.pop(0))
                        h = hg[0]
                        nc.scalar.copy(
                            out=u_all[0:O_PER_CORE, img, h * SH : (h + 1) * SH],
                            in_=ps[0:O_PER_CORE, h, 0:SH],
                        )
                        nc.vector.bn_stats(
                            out=stats[:, img * 2 + h, :],
                            in_=u_all[:, img, h * SH : (h + 1) * SH],
                        )
                if not last:
                    evac_q.append((img, ps))
                    if stats_q:
                        emit_stats(stats_q.pop(0))

            mv = small.tile([O_PER_CORE, 2], F32)
            nc.vector.bn_aggr(out=mv, in_=stats)
            stdv = small.tile([O_PER_CORE, 1], F32)
            nc.scalar.activation(
                out=stdv, in_=mv[:, 1:2], func=ACTF.Sqrt, bias=eps_sb, scale=1.0
            )
            rinv = small.tile([O_PER_CORE, 1], F32)
            nc.vector.reciprocal(out=rinv, in_=stdv)
            a_t = small.tile([O_PER_CORE, 1], F32)
            nc.vector.tensor_tensor(out=a_t, in0=rinv, in1=ngam, op=ALU.mult)
            t2 = small.tile([O_PER_CORE, 1], F32)
            nc.vector.tensor_tensor(out=t2, in0=a_t, in1=mv[:, 0:1], op=ALU.mult)
            b_t = small.tile([O_PER_CORE, 1], F32)
            nc.vector.tensor_tensor(out=b_t, in0=beta, in1=t2, op=ALU.subtract)

            # y = u*A + B, split DVE / ACT / Pool; each engine queues its own
            # chunk's output DMA so the three configs don't serialize on SP.
            nc.vector.tensor_scalar(
                out=y_sb[:, 0:3, :], in0=u_all[:, 0:3, :],
                scalar1=a_t, scalar2=b_t, op0=ALU.mult, op1=ALU.add,
            )
            nc.sync.dma_start(out=y_out[:, 0:3, :], in_=y_sb[:, 0:3, :])
            nc.scalar.activation(
                out=y_sb[:, 3:6, :], in_=u_all[:, 3:6, :],
                func=ACTF.Identity, bias=b_t, scale=a_t,
            )
            nc.scalar.dma_start(out=y_out[:, 3:6, :], in_=y_sb[:, 3:6, :])
            nc.gpsimd.tensor_scalar(
                out=y_sb[:, 6:8, :], in0=u_all[:, 6:8, :],
                scalar1=a_t, scalar2=b_t, op0=ALU.mult, op1=ALU.add,
            )
            nc.gpsimd.dma_start(out=y_out[:, 6:8, :], in_=y_sb[:, 6:8, :])
    nc.finalize()
    return nc


_NC_CACHE: dict = {}


def _get_nc() -> bass.Bass:
    if "nc" not in _NC_CACHE:
        _NC_CACHE["nc"] = _build_nc()
    return _NC_CACHE["nc"]


def _bf16(a):
    import ml_dtypes

    return np.ascontiguousarray(a).astype(ml_dtypes.bfloat16)


def _prep_x(x):
    """[8, 32, 28, 28] f32 -> (xx bf16 [8,128,1920], sxg bf16 [8,2,784]).

    xx: zero-padded to 30x32 (row stride 32), replicated into 4 partition
    blocks, bf16; first 960 columns normal, last 960 shifted left one element
    (dx=1 alignment). sxg: bf16 hi/lo split of the channel-and-tap-summed
    input windows over the min-trick taps.
    """
    xp = np.zeros((N_IMG, C_IN, HP, WP), dtype=np.float32)
    xp[:, :, 1 : 1 + HW, 1 : 1 + HW] = x
    xb1 = _bf16(xp)  # [8, 32, 30, 32]
    xb = np.tile(xb1.reshape(N_IMG, C_IN, PADN), (1, 4, 1))
    xo = np.zeros_like(xb)
    xo[:, :, : PADN - 1] = xb[:, :, 1:]
    xx = np.concatenate([xb[:, :, None, :], xo[:, :, None, :]], axis=2)
    xx = np.ascontiguousarray(xx.reshape(N_IMG, 128, 2 * PADN))

    csum = xb1.astype(np.float32).sum(axis=1)  # [8, 30, 32]
    sx = np.zeros((N_IMG, HW, HW), dtype=np.float32)
    for j in DVE_TAPS:
        dy, dx = divmod(j, 3)
        sx += csum[:, dy : dy + HW, dx : dx + HW]
    sx = sx.reshape(N_IMG, S)
    hi = _bf16(sx)
    lo = _bf16(sx - hi.astype(np.float32))
    sxg = np.ascontiguousarray(np.stack([hi, lo], axis=1))
    return xx, sxg


def _in_maps(x, W, gamma, beta):
    x = np.ascontiguousarray(x, dtype=np.float32)
    W = np.asarray(W, dtype=np.float32)
    gamma = np.asarray(gamma, dtype=np.float32)
    beta = np.asarray(beta, dtype=np.float32)
    xx, sxg = _prep_x(x)

    slot = np.arange(128) // 32  # partition -> o-slot
    gmat = (slot[:, None] == np.arange(4)[None, :]).astype(np.float32)
    pb = np.zeros((128, PB_COLS), dtype=np.float32)
    for g in range(N_GRP):
        pb[:, PB_M2G + 8 * g + 4 * g : PB_M2G + 8 * g + 4 * g + 4] = -2.0 * gmat
        pb[:, PB_G + 8 * g + 4 * g : PB_G + 8 * g + 4 * g + 4] = gmat
    pb[0:2, PB_ONES : PB_ONES + 8] = 1.0
    pb = _bf16(pb)

    maps = []
    for core in range(N_CORES):
        base = core * O_PER_CORE
        w8 = W[base : base + O_PER_CORE].reshape(N_GRP, 4, C_IN, 9)
        wt = w8.transpose(1, 2, 0, 3).reshape(128, N_GRP * 9)
        pf = np.zeros((128, PF_COLS), dtype=np.float32)
        pf[:, PF_WT : PF_WT + 18] = wt
        pf[:, PF_NWT : PF_NWT + 18] = -wt
        pf[0:O_PER_CORE, PF_NGAM] = -gamma[base : base + O_PER_CORE]
        pf[0:O_PER_CORE, PF_BETA] = beta[base : base + O_PER_CORE]
        maps.append({"xx": xx, "sxg": sxg, "pf": pf, "pb": pb})
    return maps


def _gather(results) -> np.ndarray:
    y = np.empty((N_IMG, O_TOT, HW, HW), dtype=np.float32)
    for core in range(N_CORES):
        yo = results[core]["y"]  # [o_local, img, s]
        y[:, core * O_PER_CORE : (core + 1) * O_PER_CORE] = yo.transpose(
            1, 0, 2
        ).reshape(N_IMG, O_PER_CORE, HW, HW)
    return y


def run(x, W, gamma, beta, trace=False, **trace_kwargs):
    nc = _get_nc()
    maps = _in_maps(x, W, gamma, beta)
    res = run_bass_kernel_spmd(
        nc, maps, list(range(N_CORES)), trace=trace, **trace_kwargs
    )
    return _gather(res.results), res


def kernel(x, W, gamma, beta) -> np.ndarray:
    y, _ = run(x, W, gamma, beta)
    return y
